# revision 28
# baseline (speedup 1.0000x reference)
"""Trainium2 Bass kernel for nn_EncoderAtt (input-attention LSTM encoder).

Math notes (vs the reference):
  - The attention softmax is softmax(s_x + s_hc[:,None] + ab) along the
    driving-series axis.  s_hc and ab are constant along that axis, and
    softmax is shift-invariant => a = softmax(s_x) is CONSTANT over time
    and independent of h/c.  So:
        input_weighted[b,t,n] = a[b,n] * x[b,t,n]
    and the LSTM runs over precomputable inputs.
  - s_x[b,n] = sum_t x[b,t,n] * attn_w[0, 128+t]   (contraction over t)
  - LSTM gate trick: we track H = 2*h and pre-scale W_hh by 0.5 so that
    h2 = sigma(o)*tanh(c2) = 0.5*(tanh(o/2)+1)*tanh(c2) becomes
    H2 = (tanh(o_pre/2) + 1) * tanh(c2)  -- one fused DVE op
    (scalar_tensor_tensor).  The o-gate pre-activation is halved by
    statically scaling the o columns of W_ih / W_hh / bias by 0.5.
    The DRAM output is written with a 0.5 scale during the final
    transpose copies.

Engine-assignment notes: walrus allows only one semaphore wait per
Matmult/LDWEIGHTS, so instruction placement keeps each matmul dependent
on a single engine's semaphore (PSUM->SBUF copies feeding matmuls go on
DVE; an early dummy matmul absorbs the identity-tile dependency).

Sharding: data-parallel over batch, 64 rows per core x 8 cores.
"""

import sys

import numpy as np

for _p in ("/opt/trn_rl_repo",):
    if _p not in sys.path:
        sys.path.append(_p)

import concourse.bass as bass
import concourse.bacc as bacc
import concourse.mybir as mybir
import concourse.tile as tile
from contextlib import ExitStack

FP = mybir.dt.float32
B_CORE = 64  # batch rows per core
T = 256  # timesteps
N = 128  # driving series (input size)
H = 64  # hidden size
NCORES = 8


def build_nc(use_gpsimd=False, do_scan=True, do_phase3=True, debug=False):
    nc = bacc.Bacc()

    x_dram = nc.dram_tensor("input_data", [B_CORE, T, N], FP, kind="ExternalInput")
    attn_w = nc.dram_tensor("attn_w", [1, 2 * H + T], FP, kind="ExternalInput")
    w_ih = nc.dram_tensor("W_ih", [4 * H, N], FP, kind="ExternalInput")
    w_hh = nc.dram_tensor("W_hh", [4 * H, H], FP, kind="ExternalInput")
    b_ih = nc.dram_tensor("b_ih", [4 * H], FP, kind="ExternalInput")
    b_hh = nc.dram_tensor("b_hh", [4 * H], FP, kind="ExternalInput")
    iw_dram = nc.dram_tensor(
        "input_weighted", [B_CORE, T, N], FP, kind="ExternalOutput"
    )
    enc_dram = nc.dram_tensor(
        "input_encoded", [B_CORE, T, H], FP, kind="ExternalOutput"
    )
    a_dram = nc.dram_tensor("a_bounce", [B_CORE, N], FP)
    if debug:
        dbg_sx = nc.dram_tensor("dbg_sx", [B_CORE, N], FP, kind="ExternalOutput")
        dbg_a = nc.dram_tensor("dbg_a", [B_CORE, N], FP, kind="ExternalOutput")
        dbg_row = nc.dram_tensor("dbg_row", [1, B_CORE * N], FP, kind="ExternalOutput")
        dbg_arep = nc.dram_tensor("dbg_arep", [128, 16 * N], FP, kind="ExternalOutput")

    Sig = mybir.ActivationFunctionType.Sigmoid
    Tanh = mybir.ActivationFunctionType.Tanh
    Exp = mybir.ActivationFunctionType.Exp
    Mul = mybir.AluOpType.mult
    Add = mybir.AluOpType.add

    with tile.TileContext(nc) as tc, ExitStack() as top:
        consts = top.enter_context(tc.tile_pool(name="consts", bufs=1))

        # ---------------- phase 0: constants / weight prep ----------------
        ident = consts.tile([128, 128], FP)
        nc.gpsimd.memset(ident, 0.0)
        nc.gpsimd.affine_select(
            out=ident,
            in_=ident,
            compare_op=mybir.AluOpType.not_equal,
            fill=1.0,
            base=0,
            pattern=[[-1, 128]],
            channel_multiplier=1,
        )
        ones_col = consts.tile([128, 1], FP)
        nc.vector.memset(ones_col, 1.0)

        # aw_x columns: (128, 2), col h = attn_w[0, 2H + h*128 : ...]
        awx = consts.tile([128, 2], FP)
        for h in range(2):
            nc.sync.dma_start(
                out=awx[:, h : h + 1],
                in_=attn_w[0:1, 2 * H + h * 128 : 2 * H + (h + 1) * 128],
            )

        # WihT: (N=128, 4H=256), cols = gate (i,f,g,o) x 64h
        wihT = consts.tile([N, 4 * H], FP)
        # WhhTa: (H+1=65, 256): rows 0:64 = 0.5*W_hh.T (o cols x0.25),
        # row 64 = bias (b_ih + b_hh, o part x0.5)
        whhTa = consts.tile([H + 1, 4 * H], FP)

        with ExitStack() as ph0:
            wtmp_pool = ph0.enter_context(tc.tile_pool(name="wtmp", bufs=1))
            wps_pool = ph0.enter_context(tc.tile_pool(name="wps", bufs=1, space="PSUM"))
            dummy_ps = wps_pool.tile([1, 1], FP, tag="dummy")
            # absorbs the Pool(identity) dependency so later transposes
            # carry only their DMA wait (1-wait matmul limit)
            nc.tensor.matmul(dummy_ps, ident[0:1, 0:1], ident[0:1, 0:1])
            for half in range(2):
                wt = wtmp_pool.tile([128, N], FP, tag=f"wt{half}")
                nc.sync.dma_start(out=wt, in_=w_ih[half * 128 : (half + 1) * 128, :])
                ps = wps_pool.tile([N, 128], FP, tag=f"wps{half}")
                nc.tensor.transpose(ps, wt, ident)
                nc.vector.tensor_copy(wihT[:, half * 128 : (half + 1) * 128], ps)
            for half in range(2):
                wt = wtmp_pool.tile([128, H], FP, tag=f"wt2{half}")
                nc.sync.dma_start(out=wt, in_=w_hh[half * 128 : (half + 1) * 128, :])
                ps = wps_pool.tile([H, 128], FP, tag=f"wps2{half}")
                nc.tensor.transpose(ps, wt, ident)
                # copy with 0.5 scale (H=2h trick)
                nc.vector.tensor_scalar_mul(
                    whhTa[0:H, half * 128 : (half + 1) * 128], ps, 0.5
                )
        # o columns x0.5 (sigmoid-via-tanh trick)
        nc.vector.tensor_scalar_mul(wihT[:, 192:256], wihT[:, 192:256], 0.5)
        nc.vector.tensor_scalar_mul(whhTa[0:H, 192:256], whhTa[0:H, 192:256], 0.5)

        btmp = consts.tile([1, 4 * H], FP)
        btmp2 = consts.tile([1, 4 * H], FP)
        nc.sync.dma_start(out=btmp, in_=b_ih[:])
        nc.sync.dma_start(out=btmp2, in_=b_hh[:])
        nc.vector.tensor_add(whhTa[H : H + 1, :], btmp, btmp2)
        nc.vector.tensor_scalar_mul(
            whhTa[H : H + 1, 192:256], whhTa[H : H + 1, 192:256], 0.5
        )

        # iwT: weighted input, transposed: (N=128, B_CORE*T), col = b*T + t
        iwT_pool = top.enter_context(tc.tile_pool(name="iwT", bufs=1))
        iwT = iwT_pool.tile([N, B_CORE * T], FP)

        sx_sb = consts.tile([B_CORE, N], FP)

        # ---------------- phase 1: s_x, softmax, weighting, transposes ----
        with ExitStack() as ph1:
            xpool = ph1.enter_context(tc.tile_pool(name="xsb", bufs=1))
            # col chunk (b, half) at (b*2+half)*128, partitions = t%128
            x_sb = xpool.tile([128, B_CORE * T], FP)
            sx_row = xpool.tile([1, B_CORE * N], FP, tag="sxrow")
            xs_pool = ph1.enter_context(tc.tile_pool(name="xs", bufs=4))
            sxps_pool = ph1.enter_context(
                tc.tile_pool(name="sxps", bufs=4, space="PSUM")
            )

            for b in range(B_CORE):
                for half in range(2):
                    nc.sync.dma_start(
                        out=x_sb[:, (b * 2 + half) * 128 : (b * 2 + half + 1) * 128],
                        in_=x_dram[b, half * 128 : (half + 1) * 128, :],
                    )
                sx_ps = sxps_pool.tile([1, N], FP, tag="sx")
                for half in range(2):
                    # xs = x * aw_t  (per-partition scalar premultiply, DVE)
                    xs = xs_pool.tile([128, N], FP, tag="xs")
                    nc.vector.tensor_scalar_mul(
                        xs,
                        x_sb[:, (b * 2 + half) * 128 : (b * 2 + half + 1) * 128],
                        awx[:, half : half + 1],
                    )
                    # column sum via ones-stationary matmul
                    nc.tensor.matmul(
                        sx_ps, ones_col, xs, start=(half == 0), stop=(half == 1)
                    )
                # PSUM row 0 -> partition-0 staging strip (DVE keeps matmul
                # waits single-semaphore)
                nc.vector.tensor_copy(sx_row[:, b * N : (b + 1) * N], sx_ps)
                # per-row partition scatter (a single strided scatter DMA
                # showed one-row corruption on HW)
                nc.sync.dma_start(
                    out=sx_sb[b : b + 1, :], in_=sx_row[:, b * N : (b + 1) * N]
                )

            # softmax over free dim (no max-subtract needed; |s_x| < ~6)
            p_exp = consts.tile([B_CORE, N], FP)
            ssum = consts.tile([B_CORE, 1], FP)
            nc.scalar.activation(out=p_exp, in_=sx_sb, func=Exp, accum_out=ssum)
            rec = consts.tile([B_CORE, 1], FP)
            nc.vector.reciprocal(rec, ssum)
            a_sb = consts.tile([B_CORE, N], FP)
            nc.vector.tensor_scalar_mul(a_sb, p_exp, rec)
            # bounce a to DRAM so it can be partition-broadcast on reload
            nc.sync.dma_start(out=a_dram[:, :], in_=a_sb)
            if debug:
                nc.sync.dma_start(out=dbg_sx[:, :], in_=sx_sb)
                nc.sync.dma_start(out=dbg_a[:, :], in_=a_sb)
                nc.sync.dma_start(out=dbg_row[:, :], in_=sx_row)

            # broadcast a over 128 partitions, in chunks of AB batch rows
            AB = 16
            arep_pool = ph1.enter_context(tc.tile_pool(name="arep", bufs=2))
            trp_pool = ph1.enter_context(tc.tile_pool(name="trp", bufs=3, space="PSUM"))
            for bc in range(B_CORE // AB):
                arep = arep_pool.tile([128, AB * N], FP, tag="ar")
                a_slice = a_dram[bc * AB : (bc + 1) * AB, :]
                a_bcast = bass.AP(
                    tensor=a_slice.tensor,
                    offset=a_slice.offset,
                    ap=[[0, 128]] + a_slice.ap,
                )  # (128, AB, N) partition-broadcast from DRAM
                nc.sync.dma_start(
                    out=arep.rearrange("p (b n) -> p b n", b=AB), in_=a_bcast
                )
                if debug and bc == 0:
                    nc.sync.dma_start(out=dbg_arep[:, :], in_=arep)
                for bi in range(AB):
                    b = bc * AB + bi
                    for half in range(2):
                        chunk = x_sb[
                            :, (b * 2 + half) * 128 : (b * 2 + half + 1) * 128
                        ]
                        # iw = x * a  (in-place), a broadcast across t-parts
                        nc.vector.tensor_mul(
                            chunk, chunk, arep[:, bi * N : (bi + 1) * N]
                        )
                        nc.sync.dma_start(
                            out=iw_dram[b, half * 128 : (half + 1) * 128, :],
                            in_=chunk,
                        )
                        ps = trp_pool.tile([N, 128], FP, tag="tr")
                        nc.tensor.transpose(ps, chunk, ident)
                        nc.vector.tensor_copy(
                            iwT[:, b * T + half * 128 : b * T + (half + 1) * 128],
                            ps,
                        )

        # ---------------- phase 2: LSTM scan ----------------
        # enc_T: (65, (T+1)*64): row 64 = ones; col block t holds
        # H(t-1)=2h(t-1); block 0 = zeros (h0).  col = t*64 + b.
        enc_pool = top.enter_context(tc.tile_pool(name="encT", bufs=1))
        encT = enc_pool.tile([H + 1, (T + 1) * H], FP)
        nc.vector.memset(encT[0:H, 0:H], 0.0)
        nc.vector.memset(encT[H : H + 1, :], 1.0)

        gates_pool = top.enter_context(tc.tile_pool(name="gates", bufs=5, space="PSUM"))
        spool = top.enter_context(tc.tile_pool(name="sact", bufs=3))
        vpool = top.enter_context(tc.tile_pool(name="vw", bufs=3))
        cpool = top.enter_context(tc.tile_pool(name="cstate", bufs=2))
        tr3_pool = top.enter_context(tc.tile_pool(name="tr3", bufs=3, space="PSUM"))
        opool = top.enter_context(tc.tile_pool(name="encout", bufs=3))

        c_prev = cpool.tile([H, B_CORE], FP, tag="c")
        nc.vector.memset(c_prev, 0.0)

        for t in range(T if do_scan else 0):
            gps = gates_pool.tile([H, 4 * B_CORE], FP, tag="g")
            # gate X pre-activation into col block X: W_ih part (start),
            # then 0.5*W_hh.T @ H(t-1) + bias via encT ones row (stop).
            # (one open accumulation group per bank at a time)
            for X in range(4):
                nc.tensor.matmul(
                    gps[:, X * B_CORE : (X + 1) * B_CORE],
                    wihT[:, X * H : (X + 1) * H],
                    iwT[:, t :: T],
                    start=True,
                    stop=False,
                )
                nc.tensor.matmul(
                    gps[:, X * B_CORE : (X + 1) * B_CORE],
                    whhTa[:, X * H : (X + 1) * H],
                    encT[:, t * H : (t + 1) * H],
                    start=False,
                    stop=True,
                )

            s_if = spool.tile([H, 2 * B_CORE], FP, tag="sif")
            nc.scalar.activation(out=s_if, in_=gps[:, 0 : 2 * B_CORE], func=Sig)
            t_go = spool.tile([H, 2 * B_CORE], FP, tag="tgo")
            nc.scalar.activation(
                out=t_go, in_=gps[:, 2 * B_CORE : 4 * B_CORE], func=Tanh
            )

            v = vpool.tile([H, B_CORE], FP, tag="v")
            nc.vector.tensor_mul(v, s_if[:, 0:B_CORE], t_go[:, 0:B_CORE])
            w = vpool.tile([H, B_CORE], FP, tag="w")
            weng = nc.gpsimd if use_gpsimd else nc.vector
            weng.tensor_mul(w, s_if[:, B_CORE : 2 * B_CORE], c_prev)
            c_new = cpool.tile([H, B_CORE], FP, tag="c")
            nc.vector.tensor_add(c_new, v, w)
            th = vpool.tile([H, B_CORE], FP, tag="th")
            nc.scalar.activation(out=th, in_=c_new, func=Tanh)
            # H(t) = (tanh(o/2) + 1) * tanh(c_new) = 2*h(t)
            nc.vector.scalar_tensor_tensor(
                out=encT[0:H, (t + 1) * H : (t + 2) * H],
                in0=t_go[:, B_CORE : 2 * B_CORE],
                scalar=1.0,
                in1=th,
                op0=Add,
                op1=Mul,
            )
            c_prev = c_new

        # ---------------- phase 3: transpose h out, scale 0.5 -------------
        for b in range(B_CORE if (do_phase3 and do_scan) else 0):
            for half in range(2):
                src = encT[0:H, H + half * 128 * H + b :: H]
                src = src[:, 0:128]  # (64, 128) strided cols
                ps = tr3_pool.tile([128, H], FP, tag="t3")
                nc.tensor.transpose(ps, src, ident[0:H, 0:H])
                ob = opool.tile([128, H], FP, tag="ob")
                nc.scalar.mul(out=ob, in_=ps, mul=0.5)
                nc.sync.dma_start(
                    out=enc_dram[b, half * 128 : (half + 1) * 128, :], in_=ob
                )

    nc.finalize()
    return nc


_NC_CACHE = {}


def _get_nc():
    if "nc" not in _NC_CACHE:
        _NC_CACHE["nc"] = build_nc()
    return _NC_CACHE["nc"]


def kernel(**inputs):
    from concourse.bass_utils import run_bass_kernel_spmd

    x = np.ascontiguousarray(np.asarray(inputs["input_data"], dtype=np.float32))
    attn_w = np.ascontiguousarray(np.asarray(inputs["attn_w"], dtype=np.float32))
    w_ih = np.ascontiguousarray(np.asarray(inputs["W_ih"], dtype=np.float32))
    w_hh = np.ascontiguousarray(np.asarray(inputs["W_hh"], dtype=np.float32))
    b_ih = np.ascontiguousarray(np.asarray(inputs["b_ih"], dtype=np.float32))
    b_hh = np.ascontiguousarray(np.asarray(inputs["b_hh"], dtype=np.float32))

    nc = _get_nc()
    in_maps = []
    for k in range(NCORES):
        in_maps.append(
            {
                "input_data": np.ascontiguousarray(x[k * B_CORE : (k + 1) * B_CORE]),
                "attn_w": attn_w,
                "W_ih": w_ih,
                "W_hh": w_hh,
                "b_ih": b_ih,
                "b_hh": b_hh,
            }
        )
    res = run_bass_kernel_spmd(nc, in_maps, core_ids=list(range(NCORES)))
    iw = np.concatenate([res.results[k]["input_weighted"] for k in range(NCORES)], 0)
    enc = np.concatenate([res.results[k]["input_encoded"] for k in range(NCORES)], 0)
    return iw, enc
